# revision 8
# baseline (speedup 1.0000x reference)
"""Multi-head attention Trainium2 Bass kernel (v2).

Problem: x[8,1024,768], qkv_w[2304,768], qkv_b[2304], proj_w[768,768],
proj_b[768] -> out[8,1024,768]  (12 heads, head_dim 64, softmax scale 1/8).

Sharding: data-parallel over the batch dim - one batch element per
NeuronCore, 8 cores, no collectives.

v2 structure (v1 spent ~73us of serial prologue before the first exp and
ran the attention loop PE-bound at ~95%):

  1. Prologue aimed at "first exp ~17us":
     - SWDGE (gpsimd casting DMA) queue carries ONLY x (two halves) then
       the q/k weight tiles 1-5 and proj_w; x half A is the first byte
       moved.
     - The scalar HWDGE queue carries biases + 8 f32 weight rows
       (q0/k0/v0/v1 for PE-identity transposes, v2-5 as f32 staging that
       DVE casts to bf16 for the XBAR).
     - The sync HWDGE queue (sole XBAR user; concurrent XBAR queues
       corrupt) does x halves A/B first, then v2-5, then trickles the
       remaining w/proj_w transposes as steady-state fillers.
  2. Q/K projections in fp8e4 DoubleRow (contraction 256/matmul); V /
     PV / proj stay bf16 (fp8 there feeds the output, ~1.5% rel err).
  3. Attention processes HEAD PAIRS (heads 2t / 2t+1 live in partition
     halves 0:64 / 64:128 of q/k tile t).  Per jt the four score matmuls
     are interleaved A,B,A,B: the auto-derived tile_position rows (0,0)
     and (64,0) let the two K=64 matmuls run CONCURRENTLY in the PE
     array (row-group packing), halving score time.  exp A then exp B
     keep ACT 100% busy; pv(jt-1) is software-pipelined one step behind
     the scores so the PE never waits on ACT.
  4. PSUM budget (8 banks): scores/filler ring "ps" 2x[128,1024] (4) +
     pv accumulators "o" 2x[65,1024] (4).  qk/v/proj/transpose matmuls
     allocate from the "ps" ring between score uses.
  5. Per-head epilogue unchanged from v1: ones-column denominator row,
     DRAM-bounce broadcast of the reciprocal, normalize multiply
     deferred into the next pair.
"""

import sys

if "/opt/trn_rl_repo" not in sys.path:
    sys.path.insert(0, "/opt/trn_rl_repo")

from contextlib import ExitStack

import numpy as np

import concourse.bass as bass
import concourse.tile as tile
from concourse import mybir
from concourse.bass_utils import run_bass_kernel_spmd
from concourse.masks import make_identity

F32 = mybir.dt.float32
BF16 = mybir.dt.bfloat16
FP8 = mybir.dt.float8e4
AF = mybir.ActivationFunctionType
DR = mybir.MatmulPerfMode.DoubleRow


def _split_dma_waits(nc: bass.Bass):
    """TRN2 instruction encodings hold at most 1 sync-wait (EventSemaphore: 2),
    but Tile can attach several (producer + xbar-mode serialization guards).
    Hoist all but one wait onto single-wait NoOps inserted just before on the
    same engine - same-sequencer FIFO order makes this equivalent.
    """
    for f in nc.m.functions:
        for blk in f.blocks:
            insts = blk.instructions
            i = 0
            while i < len(insts):
                inst = insts[i]
                limit = 2 if isinstance(inst, mybir.InstEventSemaphore) else 1
                if (inst.sync_info is not None
                        and len(inst.sync_info.on_wait) > limit):
                    waits = list(inst.sync_info.on_wait)
                    inst.sync_info = mybir.SyncInfo(
                        on_wait=waits[-limit:],
                        on_update=list(inst.sync_info.on_update))
                    for w in waits[:-limit]:
                        nop = mybir.InstNoOp(
                            name=nc.get_next_instruction_name(),
                            ins=[], outs=[])
                        nop.engine = inst.engine
                        nop.sync_info = mybir.SyncInfo(
                            on_wait=[w], on_update=[])
                        insts.insert(i, nop)
                        i += 1
                i += 1


B, N, C = 8, 1024, 768
H, HD = 12, 64
D3 = 3 * C  # 2304
SCALE = HD ** -0.5
NT = N // 128   # 8  token tiles
CT = C // 128   # 6  channel tiles


def build_kernel(nc: bass.Bass):
    x = nc.dram_tensor("x", [N, C], F32, kind="ExternalInput").ap()
    qkv_w = nc.dram_tensor("qkv_w", [D3, C], F32, kind="ExternalInput").ap()
    qkv_b = nc.dram_tensor("qkv_b", [D3], F32, kind="ExternalInput").ap()
    proj_w = nc.dram_tensor("proj_w", [C, C], F32, kind="ExternalInput").ap()
    proj_b = nc.dram_tensor("proj_b", [C], F32, kind="ExternalInput").ap()
    out = nc.dram_tensor("out", [N, C], F32, kind="ExternalOutput").ap()

    def bcast_ap(src: bass.AP, parts: int) -> bass.AP:
        # partition-broadcast a 1-D DRAM row: ap [[0, parts], [1, n]]
        return bass.AP(tensor=src.tensor, offset=src.offset,
                       ap=[[0, parts], *src.ap])

    with tile.TileContext(nc) as tc, ExitStack() as ctx:
        consts = ctx.enter_context(tc.tile_pool(name="consts", bufs=1))
        stage = ctx.enter_context(tc.tile_pool(name="stage", bufs=2))
        expp = ctx.enter_context(tc.tile_pool(name="expp", bufs=4))
        outp = ctx.enter_context(tc.tile_pool(name="outp", bufs=2))
        ps = ctx.enter_context(tc.tile_pool(name="ps", bufs=2, space="PSUM"))
        ps_o = ctx.enter_context(tc.tile_pool(name="ps_o", bufs=2,
                                              space="PSUM"))
        dram = ctx.enter_context(tc.tile_pool(name="dram", bufs=1,
                                              space="DRAM"))

        # ---- persistent operands -------------------------------------
        xT8 = consts.tile([128, CT, N], FP8)        # x.T   [c, n] (q/k DR)
        wT8 = consts.tile([128, CT, 2 * C], FP8)    # qkv_w.T q,k rows (DR)
        qTt = consts.tile([128, CT, N], BF16)       # q.T  [d, n] (+bias)
        kTt = consts.tile([128, CT, N], BF16)       # k.T  [d, n] (+bias)
        v_sb = consts.tile([128, NT, H, HD + 1], BF16)  # v natural + ones
        attnU = consts.tile([128, CT, N], BF16)     # attn.T (normalized
        attnT = attnU                               # in place per head)
        qkb = consts.tile([128, 2 * CT], F32)       # q,k bias per-partition
        vb_bc = consts.tile([128, C], F32)          # v bias bcast
        pjb_bc = consts.tile([128, C], F32)         # proj bias bcast
        dscratch = dram.tile([H, N], F32)           # DRAM bounce: denom
        dscratch2 = dram.tile([H, N], F32)          # DRAM bounce: 1/denom

        # ---- biases + w rows (scalar HWDGE queue; f32, no cast) ------
        nc.scalar.dma_start(out=qkb, in_=qkv_b[0:2 * C].rearrange(
            "(t p) -> p t", p=128))
        nc.scalar.dma_start(out=vb_bc, in_=bcast_ap(qkv_b[2 * C:D3], 128))
        nc.scalar.dma_start(out=pjb_bc, in_=bcast_ap(proj_b, 128))
        nc.vector.memset(v_sb[:, :, :, HD:HD + 1], 1.0)

        # preload the exp table-set before the first real exp
        dummy = stage.tile([1, 8], F32, tag="dummy", bufs=1)
        nc.scalar.activation(out=dummy, in_=qkb[0:1, 0:8], func=AF.Exp)

        # q0/k0/v0/v1 go to PE-identity transposes; v2-5 are f32 staging
        # for a DVE bf16 cast feeding the XBAR.
        w32 = {}
        for nm, roff in (("q0", 0), ("k0", C), ("v0", 2 * C),
                         ("v1", 2 * C + 128), ("v2", 2 * C + 2 * 128),
                         ("v3", 2 * C + 3 * 128), ("v4", 2 * C + 4 * 128),
                         ("v5", 2 * C + 5 * 128)):
            w32[nm] = stage.tile([128, C], F32, tag="w32", bufs=4,
                                 name=f"w32{nm}")
            nc.scalar.dma_start(out=w32[nm], in_=qkv_w[roff:roff + 128, :])
        # DVE casts f32 -> bf16 for v2-5 (XBAR input must be 2-byte)
        v_bf = {}
        for j in range(2, CT):
            v_bf[j] = stage.tile([128, C], BF16, tag="v_bf", bufs=2,
                                 name=f"v_bf{j}")
            nc.vector.tensor_copy(out=v_bf[j], in_=w32[f"v{j}"])

        # ---- SWDGE casting loads (x first - it gates everything) -----
        # x is loaded with the PERMUTED token order n = p*8 + t (contiguous
        # runs per partition -> fast SWDGE).  The permutation is consistent
        # through the whole attention pipeline and inverted by the final
        # output-DMA scatter.
        x_sb = stage.tile([128, NT, C], BF16, tag="x_sb", bufs=1)
        xv = x.rearrange("(p t) c -> p t c", t=NT)
        nc.gpsimd.dma_start(out=x_sb[:, 0:4, :], in_=xv[:, 0:4, :])
        nc.gpsimd.dma_start(out=x_sb[:, 4:8, :], in_=xv[:, 4:8, :])
        # identity for the PE transposes rides the gpsimd queue - emit it
        # before the bulk w loads so it lands early
        ident = consts.tile([128, 128], F32)
        make_identity(nc, ident)
        # q/k weight tiles 1-5 (bf16 staging for XBAR), interleaved q,k
        w_sbs = {}
        for roff in (0, C):
            w_sbs[roff] = stage.tile([128, CT, C], BF16, tag=f"w_sb{roff}",
                                     bufs=1, name=f"w{roff}")
        for j in range(1, CT):
            for roff in (0, C):
                nc.gpsimd.dma_start(
                    out=w_sbs[roff][:, j, :],
                    in_=qkv_w[roff + j * 128:roff + (j + 1) * 128, :])
        pw_sb = stage.tile([128, CT, C], BF16, tag="pw_sb", bufs=1,
                           name="pw_sb")
        nc.gpsimd.dma_start(
            out=pw_sb, in_=proj_w.rearrange("(t p) c -> p t c", p=128))

        # ---- PE identity transposes (idle PE; keeps XBAR prefix = x) --
        wTv_bf = stage.tile([128, CT, CT, 128], BF16, tag="wTv_bf", bufs=1)

        def pe_xpose(nm, dst, to_wT8):
            # transpose one 128-row w chunk via PE identity; DVE copies
            # PSUM f32 -> fp8 (q/k) or bf16 (v)
            for c0, w in ((0, 4), (4, 2)):
                pst = ps.tile([128, 512], F32, tag="ps", name="pstw")
                for k in range(w):
                    nc.tensor.transpose(
                        out=pst[:, k * 128:(k + 1) * 128],
                        in_=w32[nm][:, (c0 + k) * 128:(c0 + k + 1) * 128],
                        identity=ident)
                nc.vector.tensor_copy(
                    out=dst[:, c0:c0 + w, 0:128] if to_wT8
                    else dst[:, c0:c0 + w, :],
                    in_=pst[:, 0:w * 128].rearrange("p (c q) -> p c q", q=128))

        pe_xpose("q0", wT8[:, :, 0:128], True)
        pe_xpose("k0", wT8[:, :, C:C + 128], True)
        pe_xpose("v0", wTv_bf[:, 0], False)
        pe_xpose("v1", wTv_bf[:, 1], False)

        # ---- XBAR transposes (single sync queue) ---------------------
        def xpose(dst_ap, src_ap):
            nc.sync.dma_start_transpose(out=dst_ap, in_=src_ap)

        xT_bf = stage.tile([128, NT, CT, 128], BF16, tag="xT_bf", bufs=1)
        xT8v = xT8.rearrange("p c (t q) -> p t c q", q=128)
        xpose(xT_bf[:, 0:4].rearrange("p t c q -> p (t c) q"),
              x_sb[:, 0:4, :].rearrange("p t c -> p (t c)"))
        nc.vector.tensor_copy(out=xT8v[:, 0:4], in_=xT_bf[:, 0:4])
        xpose(xT_bf[:, 4:8].rearrange("p t c q -> p (t c) q"),
              x_sb[:, 4:8, :].rearrange("p t c -> p (t c)"))
        nc.vector.tensor_copy(out=xT8v[:, 4:8], in_=xT_bf[:, 4:8])
        for j in range(2, CT):         # v rows 2-5 via XBAR after x
            xpose(wTv_bf[:, j, :, :], v_bf[j])

        # q/k transposes for tiles 1-5 ride steady-state fillers; the
        # XBAR output lives briefly in a small ring until the fp8 cast
        pwT_bf = stage.tile([128, CT, CT, 128], BF16, tag="pwT_bf", bufs=1,
                            name="pwT_bf")

        def w_xpose(roff, j):
            # transpose w rows [roff+128j, roff+128(j+1)) and cast the
            # fresh slice to fp8 for the DoubleRow q/k matmuls; the cast
            # rides the idle gpsimd engine (its DMAs are done by then)
            wTx = stage.tile([128, CT, 128], BF16, tag="wTx", bufs=3,
                             name="wTx")
            xpose(wTx, w_sbs[roff][:, j, :])
            nc.gpsimd.tensor_copy(
                out=wT8[:, :, roff + j * 128:roff + (j + 1) * 128],
                in_=wTx)

        # ---- Q/K projection units (fp8 DoubleRow, transposed out) ----
        def qk_unit(t, is_k, ic):
            # qkvT[d', n] = wT.T @ xT for d' tile t (+C if k), n chunk ic
            woff = (C if is_k else 0) + t * 128
            dst = kTt if is_k else qTt
            psq = ps.tile([128, 512], F32, tag="ps", name="psqk")
            for k in range(3):
                nc.tensor.matmul(
                    psq,
                    lhsT=wT8[:, 2 * k:2 * k + 2, woff:woff + 128],
                    rhs=xT8[:, 2 * k:2 * k + 2, ic * 512:(ic + 1) * 512],
                    start=(k == 0), stop=(k == 2), perf_mode=DR)
            nc.vector.tensor_scalar_add(
                out=dst[:, t, ic * 512:(ic + 1) * 512], in0=psq,
                scalar1=qkb[:, CT * is_k + t:CT * is_k + t + 1])

        qk_unit(0, False, 0)
        qk_unit(0, True, 0)
        qk_unit(0, False, 1)
        qk_unit(0, True, 1)

        # ---- V projection units (bf16), natural [n, (h, d)] ----------
        def v_unit(t, chunk):
            lo, hi = (0, 512) if chunk == 0 else (512, 768)
            psv = ps.tile([128, 512], F32, tag="ps", name="psv")
            for ct in range(CT):
                nc.tensor.matmul(
                    psv[:, 0:hi - lo],
                    lhsT=xT_bf[:, t, ct, :],
                    rhs=wTv_bf[:, lo // 128:hi // 128, ct, :],
                    start=(ct == 0), stop=(ct == CT - 1))
            nc.vector.tensor_add(
                out=v_sb[:, t, lo // HD:hi // HD, 0:HD],
                in0=psv[:, 0:hi - lo].rearrange("p (h d) -> p h d", d=HD),
                in1=vb_bc[:, lo:hi].rearrange("p (h d) -> p h d", d=HD))

        # ---- attention: head pairs, pipelined pv ---------------------
        fillers = []
        prev_muls = []  # deferred normalize multiplies (previous pair)

        def epilogue(t, hh, o_ps):
            # attnU copy first (frees the PSUM accumulator), then the
            # denominator round-trip: a [1,N] DVE reciprocal is
            # single-lane (~6.5us!), so bounce the row through DRAM
            # reshaped to [128, N/128] and run it on all lanes (~0.2us).
            h = 2 * t + hh
            hb = hh * 64
            nc.vector.tensor_copy(
                out=attnU[hb:hb + 64, t, :], in_=o_ps[0:HD, :])
            den = stage.tile([1, N], F32, tag="den", bufs=1)
            nc.vector.tensor_copy(out=den, in_=o_ps[HD:HD + 1, :])
            nc.sync.dma_start(out=dscratch[h:h + 1, :], in_=den)
            den128 = stage.tile([128, N // 128], F32, tag="den128")
            nc.sync.dma_start(out=den128, in_=dscratch[h, :].rearrange(
                "(p a) -> p a", p=128))
            den128r = stage.tile([128, N // 128], F32, tag="den128r")
            nc.vector.reciprocal(out=den128r, in_=den128)
            nc.sync.dma_start(out=dscratch2[h, :].rearrange(
                "(p a) -> p a", p=128), in_=den128r)
            rbc = stage.tile([128, N], F32, tag="rbc")
            nc.sync.dma_start(out=rbc[hb:hb + 64, :],
                              in_=bcast_ap(dscratch2[h, :], 64))
            prev_muls.append(
                lambda hb=hb, t=t, rbc=rbc: nc.vector.tensor_mul(
                    out=attnT[hb:hb + 64, t, :],
                    in0=attnU[hb:hb + 64, t, :], in1=rbc[hb:hb + 64, :]))

        for t in range(CT):
            # fillers for this pair: transposes + qk for tile t+1, spare
            # v chunk-1 units and proj_w transposes
            if t < CT - 1:
                fillers.append(lambda tt=t + 1: w_xpose(0, tt))
                fillers.append(lambda tt=t + 1: w_xpose(C, tt))
                if t in (1, 2):
                    for tau in range(4 * (t - 1), 4 * t):
                        fillers.append(lambda tt=tau: v_unit(tt, 1))
                if t < 3:
                    fillers.append(lambda jj=t: xpose(
                        pwT_bf[:, jj, :, :], pw_sb[:, jj, :]))
                for is_k in (False, True):
                    for ic in range(2):
                        fillers.append(
                            lambda tt=t + 1, kk=is_k, cc=ic: qk_unit(tt, kk, cc))
            else:
                for j in range(3, CT):
                    fillers.append(lambda jj=j: xpose(
                        pwT_bf[:, jj, :, :], pw_sb[:, jj, :]))

            oA = ps_o.tile([HD + 1, N], F32, tag="o", name="o_psA")
            oB = ps_o.tile([HD + 1, N], F32, tag="o", name="o_psB")
            es = [None] * 2  # e tiles of step jt-1 (A, B)
            for jt in range(NT + 1):
                if jt < NT:
                    # scores for both heads, interleaved A,B so the two
                    # K=64 matmuls pack into PE row groups (0,0)/(64,0)
                    sA = ps.tile([128, N], F32, tag="ps", name="sA")
                    sB = ps.tile([128, N], F32, tag="ps", name="sB")
                    for ic in range(2):
                        for hb, s in ((0, sA), (64, sB)):
                            nc.tensor.matmul(
                                s[:, ic * 512:(ic + 1) * 512],
                                lhsT=kTt[hb:hb + 64, t,
                                         jt * 128:(jt + 1) * 128],
                                rhs=qTt[hb:hb + 64, t,
                                        ic * 512:(ic + 1) * 512],
                                start=True, stop=True)
                    eA = expp.tile([128, N], BF16, tag="e", name="eA")
                    eB = expp.tile([128, N], BF16, tag="e", name="eB")
                    nc.scalar.activation(out=eA, in_=sA, func=AF.Exp,
                                         scale=SCALE)
                    nc.scalar.activation(out=eB, in_=sB, func=AF.Exp,
                                         scale=SCALE)
                    if t == 0:
                        v_unit(jt, 0)
                if jt > 0:
                    # pv one step behind: e(jt-1) is ready, PE never
                    # stalls on ACT
                    for hh, o in ((0, oA), (1, oB)):
                        for ic in range(2):
                            nc.tensor.matmul(
                                o[:, ic * 512:(ic + 1) * 512],
                                lhsT=v_sb[:, jt - 1, 2 * t + hh, :],
                                rhs=es[hh][:, ic * 512:(ic + 1) * 512],
                                start=(jt == 1), stop=(jt == NT))
                if jt < NT:
                    es = [eA, eB]
                    # drain deferred normalizes from the previous pair
                    if prev_muls and jt < 2:
                        prev_muls.pop(0)()
                    if fillers and (t > 0 or jt >= 4):
                        fillers.pop(0)()
                        if len(fillers) > 2 * (NT - 1 - jt):
                            fillers.pop(0)()

            epilogue(t, 0, oA)
            epilogue(t, 1, oB)

        while fillers:
            fillers.pop(0)()
        while prev_muls:
            prev_muls.pop(0)()

        # ---- output projection (bf16) --------------------------------
        for t in range(NT):
            osb = outp.tile([128, C], F32, tag="osb")
            for lo, hi in ((0, 512), (512, 768)):
                psp = ps.tile([128, 512], F32, tag="ps", name="psp")
                for ct in range(CT):
                    nc.tensor.matmul(
                        psp[:, 0:hi - lo],
                        lhsT=attnT[:, ct, t * 128:(t + 1) * 128],
                        rhs=pwT_bf[:, lo // 128:hi // 128, ct, :],
                        start=(ct == 0), stop=(ct == CT - 1))
                nc.vector.tensor_add(
                    out=osb[:, lo:hi], in0=psp[:, 0:hi - lo],
                    in1=pjb_bc[:, lo:hi])
            eng = nc.sync if t % 2 == 0 else nc.scalar
            eng.dma_start(out=out.rearrange("(p t) c -> p t c", t=NT)[:, t, :],
                          in_=osb)

    _split_dma_waits(nc)
    return nc


_NC_CACHE = None


def _get_nc():
    global _NC_CACHE
    if _NC_CACHE is None:
        _NC_CACHE = build_kernel(
            bass.Bass("TRN2", target_bir_lowering=False, debug=False))
    return _NC_CACHE


def kernel(**inputs: np.ndarray) -> np.ndarray:
    nc = _get_nc()
    x = np.ascontiguousarray(inputs["x"], dtype=np.float32)
    shared = {
        "qkv_w": np.ascontiguousarray(inputs["qkv_w"], dtype=np.float32),
        "qkv_b": np.ascontiguousarray(inputs["qkv_b"], dtype=np.float32),
        "proj_w": np.ascontiguousarray(inputs["proj_w"], dtype=np.float32),
        "proj_b": np.ascontiguousarray(inputs["proj_b"], dtype=np.float32),
    }
    in_maps = [{"x": x[b], **shared} for b in range(B)]
    res = run_bass_kernel_spmd(nc, in_maps, core_ids=list(range(B)))
    return np.stack([r["out"] for r in res.results]).astype(np.float32)


if __name__ == "__main__":
    from reference import setup_inputs, reference

    inputs = {k: np.asarray(v) for k, v in setup_inputs().items()}
    got = kernel(**inputs)
    exp = np.asarray(reference(**inputs))
    err = np.abs(got - exp)
    print("abs err max:", err.max(), "ref absmax:", np.abs(exp).max())
    print("rel(absmax):", err.max() / np.abs(exp).max())
